# revision 45
# baseline (speedup 1.0000x reference)
"""Trainium2 Bass kernel for nn_Attention_18726057410905.

Multi-head causal attention: B=8, S=1024, D=768, N=12 heads, H=64.
Sharding: data-parallel over batch -- core b computes batch element b.
No collectives.

v2: all operands are pre-laid-out on the HOST into their exact SBUF
images and uploaded as bf16 (halves HBM traffic, kills the on-chip
x-transposes / casts / staging DMAs of v1):
  xt   [128, 6, 1024]  x^T, partition = d%128, free = (d//128, s)
  wqk  [128, 6, 2, 6, 128]  [dp, pair, q/k, dt, 2*64 packed heads]
  wv   [128, 6, 768]   [dp, dt, (n h)]
  wo   [128, 6, 768]   [2 packed heads * 64, pair, e]
  bqk  [128, 2, 6]     packed-head-major Q/K biases
  bv/bo [768]          flat; broadcast-DMA'd to [128, .] on chip

Per-core dataflow (matmul inputs bf16, fp32 PSUM accumulation):
  Q^T,K^T [2*64h, s] per head-pair (W stationary, xt moving)
  V_aug [s, n, 128]  natural layout + 64-wide ones block (cols 64:128)
  S^T   [k-tile 128, 2 halves x 512q] -- one 2-bank PSUM tile per k-tile,
        2 heads row-packed on the PE (K=64 contraction, concurrent via
        row tiling)
  P^T   = exp(S^T/8) via one ACT activation per k-tile; triangular mask
          (DVE) on diagonal tiles only; fully-masked tiles never computed
  z_aug^T [128, q] = sum_k V_aug.T @ P^T; rows 64:128 hold the softmax
        denominators replicated by the ones block (broadcast for free)
  z^T normalized with reciprocal_approx_fast + multiply
  out   [q, e] = z^T.T @ W_O + b_O

DMA plan: xt split across the sync and scalar HW-DGE queues per d-tile;
wqk pair 0 leads the scalar queue, pairs 1-5 + wo follow the xt chunks;
wv + biases ride the gpsimd SW-DGE queue.  Out tiles go back on sync.

Pipelining: PV matmuls trail S^T/exp by LOOKAHEAD k-tiles; the next pair's
Q/K projection matmuls (and, for the last pair, the output projection) are
drip-fed into the attention stream as PE filler so the in-order PE never
idles on the ACT exp stream.
"""

from contextlib import ExitStack

import numpy as np
import ml_dtypes

import concourse.bass as bass
import concourse.tile as tile
from concourse import bacc, mybir
from concourse.bass_utils import run_bass_kernel_spmd
from concourse.masks import make_upper_triangular

B, S, D, N, H = 8, 1024, 768, 12, 64
P = 128
N_CORES = 8
DT = D // P          # 6 d-tiles
NPAIR = N // 2       # 6 head pairs
QB = 512             # q-block width
SB = S // QB         # 2 q/s blocks
KT = S // P          # 8 k/s tiles
EB = 384             # e-block width for the output projection
LOOKAHEAD = 6        # k-tiles of PV deferral (keeps PE fed while ACT exps)
BF16 = mybir.dt.bfloat16
F32 = mybir.dt.float32
AF = mybir.ActivationFunctionType
ALU = mybir.AluOpType
NPBF16 = ml_dtypes.bfloat16
FP8 = mybir.dt.float8e4
NPFP8 = mybir.dt.np(FP8)
DR = mybir.MatmulPerfMode.DoubleRow
DTP = DT // 2        # 3 doubled d-tiles for the fp8 DoubleRow QK projection
WSCALE = 64.0        # fp8 weight prescale (keeps W_Q/W_K out of e4m3 subnormals)

# mechanism toggles
ACT_QEVAC = False    # Q^T evac on ACT via activation Identity+bias
PSUM_RECIP = False   # reciprocal_approx_fast reads denominators from PSUM
DVE_MASK = False     # triangular mask on DVE instead of gpsimd


def _build_nc():
    nc = bacc.Bacc(
        "TRN2", target_bir_lowering=False, debug=False, num_devices=N_CORES
    )
    xt_d = nc.dram_tensor("xt", [P, DT, S], BF16, kind="ExternalInput").ap()
    x8_d = nc.dram_tensor("x8", [P, DTP, 2, S], FP8, kind="ExternalInput").ap()
    wqk_d = nc.dram_tensor("wqk8", [P, NPAIR, 2, DTP, 2, P], FP8, kind="ExternalInput").ap()
    wv_d = nc.dram_tensor("wv", [P, DT, N * H], BF16, kind="ExternalInput").ap()
    wo_d = nc.dram_tensor("wo", [P, NPAIR, D], BF16, kind="ExternalInput").ap()
    bqk_d = nc.dram_tensor("bqk", [P, 2, NPAIR], F32, kind="ExternalInput").ap()
    bv_d = nc.dram_tensor("bv", [N * H], F32, kind="ExternalInput").ap()
    bo_d = nc.dram_tensor("bo", [D], F32, kind="ExternalInput").ap()
    out_d = nc.dram_tensor("out", [S, D], BF16, kind="ExternalOutput").ap()

    with tile.TileContext(nc) as tc, ExitStack() as ctx:
        _body(ctx, tc, xt_d, x8_d, wqk_d, wv_d, wo_d, bqk_d, bv_d, bo_d, out_d)
    nc.compile()
    return nc


def _body(ctx, tc, xt_d, x8_d, wqk_d, wv_d, wo_d, bqk_d, bv_d, bo_d, out_d):
    nc = tc.nc
    const = ctx.enter_context(tc.tile_pool(name="const", bufs=1))
    ppool = ctx.enter_context(tc.tile_pool(name="ppool", bufs=8))
    spool = ctx.enter_context(tc.tile_pool(name="spool", bufs=4))
    opool = ctx.enter_context(tc.tile_pool(name="opool", bufs=4))
    ps_mm = ctx.enter_context(tc.tile_pool(name="ps_mm", bufs=3, space="PSUM"))
    ps_pj = ps_mm
    ps_z = ctx.enter_context(tc.tile_pool(name="ps_z", bufs=2, space="PSUM"))

    # --- engine warmups ----------------------------------------------------
    # DVE pays ~11us on its first real op; ACT pays a ~2.7us exp-table load.
    # Absorb both at t=0, concurrent with the input DMAs.
    warm = const.tile([1, 8], F32, tag="warm")
    nc.vector.memset(warm[:], 1.0)
    warmp = ps_z.tile([1, 8], F32, tag="z", name="warmp")
    nc.vector.tensor_copy(warmp[:], warm[:])
    warmb = const.tile([1, 8], BF16, tag="warmb")
    nc.vector.tensor_copy(warmb[:], warmp[:])  # preload DVE psum-read CAST path
    nc.scalar.activation(warm[:], warm[:], AF.Exp, scale=1.0)

    # --- constants ---------------------------------------------------------
    # trimask[r, c] = 1 if r <= c else 0 (keep k <= q in [k, q] layout)
    trimask = const.tile([P, P], BF16, tag="trimask")
    make_upper_triangular(nc, trimask[:], val=1.0, diag=True)

    # --- input DMAs --------------------------------------------------------
    xt = const.tile([P, DT, S], BF16, tag="xt")
    x8 = const.tile([P, DTP, 2, S], FP8, tag="x8")
    wqk = const.tile([P, NPAIR, 2, DTP, 2, P], FP8, tag="wqk8")
    wv_sb = const.tile([P, DT, N * H], BF16, tag="wv")
    wo_sb = const.tile([P, NPAIR, D], BF16, tag="wo")
    bqk_sb = const.tile([P, 2, NPAIR], F32, tag="bqk")
    bv_rep = const.tile([P, N * H], F32, tag="bvrep")
    bo_rep = const.tile([P, D], F32, tag="borep")

    # gpsimd SW queue: biases and the first two wv chunks early; the bulk
    # of wv waits on a 4-byte bounce read of the last xt chunk so the
    # critical x8/xt uploads own the oversubscribed startup HBM bandwidth
    # (x8+wqk0+xt+wv together need ~470 GB/s of the ~360 GB/s core share)
    nc.gpsimd.dma_start(bqk_sb[:], bqk_d)
    nc.gpsimd.dma_start(
        bv_rep[:], bv_d[None, :].to_broadcast((P, N * H))
    )
    for dt in range(2):
        nc.gpsimd.dma_start(wv_sb[:, dt, :], wv_d[:, dt, :])
    nc.gpsimd.dma_start(bo_rep[:], bo_d[None, :].to_broadcast((P, D)))

    # scalar HW queue: pair-0 weights + last x8 chunk lead, then odd xt
    # chunks, then the remaining weights
    nc.scalar.dma_start(wqk[:, 0], wqk_d[:, 0])
    nc.scalar.dma_start(x8[:, 2], x8_d[:, 2])
    for dt in (1, 3, 5):
        nc.scalar.dma_start(xt[:, dt, :], xt_d[:, dt, :])
    for pr in range(1, NPAIR):
        nc.scalar.dma_start(wqk[:, pr], wqk_d[:, pr])
    nc.scalar.dma_start(wo_sb[:], wo_d)

    # sync HW queue: first x8 chunks, then even xt chunks (out tiles ride
    # this queue later)
    for dtp in (0, 1):
        nc.sync.dma_start(x8[:, dtp], x8_d[:, dtp])
    for dt in (0, 2, 4):
        nc.sync.dma_start(xt[:, dt, :], xt_d[:, dt, :])

    # bounce gate + deferred wv bulk (emitted after the xt DMAs so the
    # Tile dependency on xt's last chunk exists)
    xgate = const.tile([1, 1], BF16, tag="xgate")
    nc.gpsimd.dma_start(xgate[:], xt[0:1, DT - 1, 0:1])
    for dt in range(2, DT):
        nc.gpsimd.dma_start(wv_sb[:, dt, :], wv_d[:, dt, :])

    # --- persistent attention tiles ---------------------------------------
    qT = const.tile([P, NPAIR, S], BF16, tag="qT")
    kT = const.tile([P, NPAIR, S], BF16, tag="kT")
    zT = const.tile([P, NPAIR, S], BF16, tag="zT")
    # V_aug: the 64-wide ones block makes the PV matmul replicate the softmax
    # denominators into PSUM partitions 64:128 -- broadcast for free.  Only
    # the ones region needs the memset; v_proj overwrites the value region.
    v_aug = const.tile([P, KT, N, 2 * H], BF16, tag="vaug")
    nc.vector.memset(v_aug[:, :, :, H : 2 * H], 1.0)

    def qk_proj_steps(pr):
        # Q^T (bank 0) and K^T (bank 1) of one 2-bank psum tile, as a list of
        # single-matmul closures so the pair-(pr) projection can be drip-fed
        # into pair-(pr-1)'s attention stream as PE filler work.  Q evacuates
        # on ACT (activation+bias), K on DVE, splitting the evac load.
        steps = []
        for sb_i in range(SB):
            box = {}

            def mk(dtp, half, sb_i=sb_i, box=box):
                def go():
                    if "t" not in box:
                        box["t"] = ps_pj.tile(
                            [P, 2 * QB], F32, tag="mm", name=f"pqk_{pr}_{sb_i}"
                        )
                    pqk = box["t"]
                    nc.tensor.matmul(
                        pqk[:, half * QB : (half + 1) * QB],
                        lhsT=wqk[:, pr, half, dtp],
                        rhs=x8[:, dtp, :, bass.ts(sb_i, QB)],
                        start=(dtp == 0),
                        stop=(dtp == DTP - 1),
                        perf_mode=DR,
                    )
                    if half == 1 and dtp == DTP - 1:
                        if ACT_QEVAC:
                            nc.scalar.activation(
                                qT[:, pr, bass.ts(sb_i, QB)], pqk[:, 0:QB],
                                AF.Identity, bias=bqk_sb[:, 0, pr : pr + 1],
                                scale=1.0 / WSCALE,
                            )
                        else:
                            nc.vector.tensor_scalar(
                                qT[:, pr, bass.ts(sb_i, QB)], pqk[:, 0:QB],
                                1.0 / WSCALE, bqk_sb[:, 0, pr : pr + 1],
                                ALU.mult, ALU.add,
                            )
                        nc.vector.tensor_scalar(
                            kT[:, pr, bass.ts(sb_i, QB)], pqk[:, QB : 2 * QB],
                            1.0 / WSCALE, bqk_sb[:, 1, pr : pr + 1],
                            ALU.mult, ALU.add,
                        )

                return go

            for half in range(2):
                for dtp in range(DTP):
                    steps.append(mk(dtp, half))
        return steps

    def qk_proj_sb(pr, sb_i):
        for s in qk_proj_steps(pr)[sb_i * 2 * DTP : (sb_i + 1) * 2 * DTP]:
            s()

    # --- V projection: dt-major, two 4-s-tile phases ----------------------
    # 8 concurrent PSUM accumulation groups per phase (3 ps_mm tiles hosting
    # 2 384-wide groups each + 2 ps_z tiles hosting 1), so the first matmuls
    # need only wv[:, 0] off the wire instead of the whole weight.
    def v_proj_phase(stiles, cb=None):
        # 2 ps_mm + 2 ps_z tiles per phase: one ps_mm ring slot stays free
        # so `cb` can interleave pair-0 S^T tiles (their exps run on the
        # otherwise-idle ACT during the V projection)
        n_groups = 2 * len(stiles)
        n_mm = (n_groups - 2 + 1) // 2
        tiles = [
            ps_mm.tile([P, 2 * QB], F32, tag="mm", name=f"pv{stiles[0]}_{i}")
            for i in range(n_mm)
        ]
        ztiles = [
            ps_z.tile([P, QB], F32, tag="z", name=f"pvz{stiles[0]}_{i}")
            for i in range(2)
        ]
        regions = []
        for g in range(n_groups):
            st, blk = stiles[g // 2], g % 2
            if g < 2 * n_mm:
                reg = tiles[g // 2][:, (g % 2) * QB : (g % 2) * QB + EB]
            else:
                reg = ztiles[g - 2 * n_mm][:, :EB]
            regions.append((reg, st, blk))
        for dt in range(DT):
            for reg, st, blk in regions:
                nc.tensor.matmul(
                    reg,
                    lhsT=xt[:, dt, bass.ts(st, P)],
                    rhs=wv_sb[:, dt, bass.ts(blk, EB)],
                    start=(dt == 0),
                    stop=(dt == DT - 1),
                )
            if cb is not None and dt == 3:
                cb()
        for reg, st, blk in regions:
            nc.vector.tensor_tensor(
                v_aug[:, st, bass.ts(blk, 6), 0:H],
                reg.rearrange("p (n h) -> p n h", h=H),
                bv_rep[:, bass.ts(blk, EB)].rearrange("p (n h) -> p n h", h=H),
                ALU.add,
            )

    def o_proj_steps(qts, alt_pool=False):
        # out[q, e] = z^T.T @ W_O + b_O, as single-matmul closures
        steps = []
        for qt in qts:
            for eb in range(D // EB):
                box = {}

                def mk(pr, qt=qt, eb=eb, box=box):
                    def go():
                        if "t" not in box:
                            # tail-only: alternate into the attention z-pool
                            # (free after the last pair) for deeper rotation
                            if alt_pool and (2 * qt + eb) % 2:
                                box["t"] = ps_z.tile(
                                    [P, QB], F32, tag="z", name=f"po_{qt}_{eb}"
                                )
                            else:
                                box["t"] = ps_mm.tile(
                                    [P, 2 * QB], F32, tag="mm", name=f"po_{qt}_{eb}"
                                )
                        po = box["t"]
                        nc.tensor.matmul(
                            po[:, :EB],
                            lhsT=zT[:, pr, bass.ts(qt, P)],
                            rhs=wo_sb[:, pr, bass.ts(eb, EB)],
                            start=(pr == 0),
                            stop=(pr == NPAIR - 1),
                        )
                        if pr == NPAIR - 1:
                            ot = opool.tile([P, EB], BF16, tag="ot")
                            nc.vector.tensor_tensor(
                                ot[:], po[:, :EB], bo_rep[:, bass.ts(eb, EB)],
                                ALU.add,
                            )
                            oeng = (nc.sync, nc.scalar, nc.gpsimd)[(2 * qt + eb) % 3]
                            oeng.dma_start(
                                out_d[bass.ts(qt, P), bass.ts(eb, EB)], ot[:]
                            )

                    return go

                for pr in range(NPAIR):
                    steps.append(mk(pr))
        return steps

    o_first = o_proj_steps(range(4))  # q-tiles 0-3: fills attn(last, j=1)
    o_idx = [0]

    # --- attention block machinery ----------------------------------------

    class Block:
        def __init__(self, pr, j):
            self.pr, self.j = pr, j
            self.n_kt = 4 * (j + 1)
            self.pz = None
            self.pts = {}
            self.normed = False

        def alloc_pz(self):
            if self.pz is None:
                self.pz = [
                    ps_z.tile([P, QB], F32, tag="z", name=f"z_{self.pr}_{self.j}_{h}")
                    for h in range(2)
                ]

        def emit_st(self, i):
            # S^T for both halves into one 2-bank tile; exp; mask
            pr, j = self.pr, self.j
            q_off = max(0, (i - 4 * j) * P)
            ps = ps_mm.tile([P, 2 * QB], F32, tag="mm", name=f"s_{pr}_{j}_{i}")
            for half in range(2):
                lo, hi = 64 * half, 64 * half + 64
                nc.tensor.matmul(
                    ps[:, half * QB + q_off : (half + 1) * QB],
                    lhsT=kT[lo:hi, pr, bass.ts(i, P)],
                    rhs=qT[lo:hi, pr, j * QB + q_off : (j + 1) * QB],
                    start=True,
                    stop=True,
                )
            pT = ppool.tile([P, 2, QB], BF16, tag="pT")
            ps3 = ps.rearrange("p (h q) -> p h q", h=2)
            nc.scalar.activation(
                pT[:, :, q_off:], ps3[:, :, q_off:], AF.Exp, scale=0.125
            )
            if i >= 4 * j:  # diagonal tile: triangular mask, both halves
                mask_eng = nc.vector if DVE_MASK else nc.gpsimd
                mask_eng.tensor_tensor(
                    pT[:, :, q_off : q_off + P],
                    pT[:, :, q_off : q_off + P],
                    trimask[:, None, :].to_broadcast((P, 2, P)),
                    ALU.mult,
                )
            self.pts[i] = pT

        def emit_pv(self, i):
            pr, j = self.pr, self.j
            self.alloc_pz()
            q_off = max(0, (i - 4 * j) * P)
            for half in range(2):
                n = 2 * pr + half
                nc.tensor.matmul(
                    self.pz[half][:, q_off:],
                    lhsT=v_aug[:, i, n, :],
                    rhs=self.pts[i][:, half, q_off:],
                    start=(i == 0),
                    stop=(i == self.n_kt - 1),
                )
            del self.pts[i]

        def emit_norm(self):
            # normalize z and store z^T (PSUM rows 64:128 hold the
            # denominators replicated by the ones block)
            pr, j = self.pr, self.j
            for half in range(2):
                lo, hi = 64 * half, 64 * half + 64
                if PSUM_RECIP:
                    den = self.pz[half][H : 2 * H, :]
                else:
                    sm = spool.tile([64, QB], F32, tag="sm")
                    nc.vector.tensor_copy(sm[:], self.pz[half][H : 2 * H, :])
                    den = sm[:]
                rc = spool.tile([64, QB], F32, tag="rc")
                nc.vector.reciprocal_approx_fast(rc[:], den)
                nc.vector.tensor_mul(
                    zT[lo:hi, pr, bass.ts(j, QB)], self.pz[half][0:H, :], rc[:]
                )
            self.normed = True

    # --- emission: qk0 | v phases (pair-0 S^T/exp interleaved) | pairs ----
    # Pair-0 j=0's S^T tiles ride the free ps_mm ring slot inside the V
    # projection: their exps run on the otherwise-idle ACT, so pair 0
    # opens with its P^T tiles already computed.
    qk_proj_sb(0, 0)
    qk_proj_sb(0, 1)
    b00 = Block(0, 0)
    st_cnt = [0]

    def _stcb():
        if st_cnt[0] < b00.n_kt:
            b00.emit_st(st_cnt[0])
            st_cnt[0] += 1

    v_proj_phase((0, 1, 2), _stcb)
    v_proj_phase((3, 4, 5), _stcb)
    v_proj_phase((6, 7), _stcb)
    _stcb()  # remaining pair-0 j=0 tile

    # v5 emission: pair-major, per-block loop, drip-fed filler
    for pr in range(NPAIR):
        last = pr + 1 >= NPAIR
        fill = qk_proj_steps(pr + 1) if not last else None
        fill_i = [0]

        def emit_fill(k=1):
            for _ in range(k):
                if fill is not None:
                    if fill_i[0] < len(fill):
                        fill[fill_i[0]]()
                        fill_i[0] += 1
                elif cur_j[0] == 1 and o_idx[0] < len(o_first):
                    o_first[o_idx[0]]()
                    o_idx[0] += 1

        cur_j = [0]
        for j in range(SB):
            cur_j[0] = j
            b = b00 if (pr == 0 and j == 0) else Block(pr, j)
            n_kt = b.n_kt
            for i in range(n_kt):
                if not (b is b00 and i < st_cnt[0]):
                    b.emit_st(i)
                emit_fill(2 if i < 2 else 1)
                if i >= LOOKAHEAD:
                    b.emit_pv(i - LOOKAHEAD)
                    emit_fill()
            final = pr == NPAIR - 1 and j == SB - 1
            for i in range(max(0, n_kt - LOOKAHEAD), n_kt):
                b.emit_pv(i)
                if not final:
                    emit_fill()
            b.emit_norm()

    # --- output projection leftovers -----------------------------------
    for s in o_first[o_idx[0] :]:
        s()
    for s in o_proj_steps(range(4, KT), alt_pool=True):
        s()


_CACHE = {}


def get_nc():
    if "nc" not in _CACHE:
        _CACHE["nc"] = _build_nc()
    return _CACHE["nc"]


def _prep_shared(W_Q, W_K, W_V, W_O, b_Q, b_K, b_V, b_O):
    W_Q = np.asarray(W_Q, np.float32)
    W_K = np.asarray(W_K, np.float32)
    W_V = np.asarray(W_V, np.float32)
    W_O = np.asarray(W_O, np.float32)
    # wqk8 [dp, pr, half, dtp, j, a*64+h], prescaled by WSCALE
    def qk_img(W):
        return (W * WSCALE).reshape(NPAIR, 2, DTP, 2, P, H).transpose(4, 0, 2, 3, 1, 5)
    wqk = np.stack([qk_img(W_Q), qk_img(W_K)], axis=2).reshape(
        P, NPAIR, 2, DTP, 2, P
    ).astype(NPFP8)
    wv = np.ascontiguousarray(
        W_V.reshape(N, DT, P, H).transpose(2, 1, 0, 3).reshape(P, DT, N * H)
    ).astype(NPBF16)
    wo = np.ascontiguousarray(
        W_O.reshape(NPAIR, 2, H, D).transpose(1, 2, 0, 3).reshape(P, NPAIR, D)
    ).astype(NPBF16)
    def b_img(b):
        return np.asarray(b, np.float32).reshape(NPAIR, 2, H).transpose(1, 2, 0).reshape(P, NPAIR)
    bqk = np.ascontiguousarray(
        np.stack([b_img(b_Q), b_img(b_K)], axis=1)
    ).astype(np.float32)
    return {
        "wqk8": np.ascontiguousarray(wqk),
        "wv": wv,
        "wo": wo,
        "bqk": bqk,
        "bv": np.ascontiguousarray(np.asarray(b_V, np.float32).reshape(N * H)),
        "bo": np.ascontiguousarray(np.asarray(b_O, np.float32)),
    }


def _prep_x(xb):
    # xt: [1024, 768] f32 -> [128, 6, 1024] bf16 (partition = d%128)
    # x8: same data as fp8, d-tiles pair-interleaved for DoubleRow
    xT = xb.T.reshape(DT, P, S)
    xt = np.ascontiguousarray(xT.transpose(1, 0, 2)).astype(NPBF16)
    x8 = np.ascontiguousarray(
        xT.reshape(DTP, 2, P, S).transpose(2, 0, 1, 3)
    ).astype(NPFP8)
    return {"xt": xt, "x8": x8}


def kernel(normalized_resid_pre, W_Q, W_K, W_V, W_O, b_Q, b_K, b_V, b_O, **kw):
    x = np.asarray(normalized_resid_pre, dtype=np.float32)
    shared = _prep_shared(W_Q, W_K, W_V, W_O, b_Q, b_K, b_V, b_O)
    in_maps = [dict(shared, **_prep_x(x[b])) for b in range(B)]
    nc = get_nc()
    res = run_bass_kernel_spmd(nc, in_maps, core_ids=list(range(N_CORES)))
    return np.stack(
        [np.asarray(res.results[b]["out"], np.float32) for b in range(B)], axis=0
    )


# revision 47
# speedup vs baseline: 1.1636x; 1.1636x over previous
"""Trainium2 Bass kernel for nn_Attention_18726057410905.

Multi-head causal attention: B=8, S=1024, D=768, N=12 heads, H=64.
Sharding: data-parallel over batch -- core b computes batch element b.
No collectives.

v2: all operands are pre-laid-out on the HOST into their exact SBUF
images and uploaded as bf16 (halves HBM traffic, kills the on-chip
x-transposes / casts / staging DMAs of v1):
  xt   [128, 6, 1024]  x^T, partition = d%128, free = (d//128, s)
  wqk  [128, 6, 2, 6, 128]  [dp, pair, q/k, dt, 2*64 packed heads]
  wv   [128, 6, 768]   [dp, dt, (n h)]
  wo   [128, 6, 768]   [2 packed heads * 64, pair, e]
  bqk  [128, 2, 6]     packed-head-major Q/K biases
  bv/bo [768]          flat; broadcast-DMA'd to [128, .] on chip

Per-core dataflow (matmul inputs bf16, fp32 PSUM accumulation):
  Q^T,K^T [2*64h, s] per head-pair (W stationary, xt moving)
  V_aug [s, n, 128]  natural layout + 64-wide ones block (cols 64:128)
  S^T   [k-tile 128, 2 halves x 512q] -- one 2-bank PSUM tile per k-tile,
        2 heads row-packed on the PE (K=64 contraction, concurrent via
        row tiling)
  P^T   = exp(S^T/8) via one ACT activation per k-tile; triangular mask
          (DVE) on diagonal tiles only; fully-masked tiles never computed
  z_aug^T [128, q] = sum_k V_aug.T @ P^T; rows 64:128 hold the softmax
        denominators replicated by the ones block (broadcast for free)
  z^T normalized with reciprocal_approx_fast + multiply
  out   [q, e] = z^T.T @ W_O + b_O

DMA plan: xt split across the sync and scalar HW-DGE queues per d-tile;
wqk pair 0 leads the scalar queue, pairs 1-5 + wo follow the xt chunks;
wv + biases ride the gpsimd SW-DGE queue.  Out tiles go back on sync.

Pipelining: PV matmuls trail S^T/exp by LOOKAHEAD k-tiles; the next pair's
Q/K projection matmuls (and, for the last pair, the output projection) are
drip-fed into the attention stream as PE filler so the in-order PE never
idles on the ACT exp stream.
"""

from contextlib import ExitStack

import numpy as np
import ml_dtypes

import concourse.bass as bass
import concourse.tile as tile
from concourse import bacc, mybir
from concourse.bass_utils import run_bass_kernel_spmd
from concourse.masks import make_upper_triangular

B, S, D, N, H = 8, 1024, 768, 12, 64
P = 128
N_CORES = 8
DT = D // P          # 6 d-tiles
NPAIR = N // 2       # 6 head pairs
QB = 512             # q-block width
SB = S // QB         # 2 q/s blocks
KT = S // P          # 8 k/s tiles
EB = 384             # e-block width for the output projection
LOOKAHEAD = 6        # k-tiles of PV deferral (keeps PE fed while ACT exps)
BF16 = mybir.dt.bfloat16
F32 = mybir.dt.float32
AF = mybir.ActivationFunctionType
ALU = mybir.AluOpType
NPBF16 = ml_dtypes.bfloat16
FP8 = mybir.dt.float8e4
NPFP8 = mybir.dt.np(FP8)
DR = mybir.MatmulPerfMode.DoubleRow
DTP = DT // 2        # 3 doubled d-tiles for the fp8 DoubleRow QK projection
WSCALE = 64.0        # fp8 weight prescale (keeps W_Q/W_K out of e4m3 subnormals)

# mechanism toggles
ACT_QEVAC = False    # Q^T evac on ACT via activation Identity+bias
PSUM_RECIP = False   # reciprocal_approx_fast reads denominators from PSUM
DVE_MASK = False     # triangular mask on DVE instead of gpsimd


def _build_nc():
    nc = bacc.Bacc(
        "TRN2", target_bir_lowering=False, debug=False, num_devices=N_CORES
    )
    xt_d = nc.dram_tensor("xt", [P, DT, S], BF16, kind="ExternalInput").ap()
    x8_d = nc.dram_tensor("x8", [P, DTP, 2, S], FP8, kind="ExternalInput").ap()
    wqk_d = nc.dram_tensor("wqk8", [P, NPAIR, 2, DTP, 2, P], FP8, kind="ExternalInput").ap()
    wv_d = nc.dram_tensor("wv", [P, DT, N * H], BF16, kind="ExternalInput").ap()
    wo_d = nc.dram_tensor("wo", [P, NPAIR, D], BF16, kind="ExternalInput").ap()
    bqk_d = nc.dram_tensor("bqk", [P, 2, NPAIR], F32, kind="ExternalInput").ap()
    bv_d = nc.dram_tensor("bv", [N * H], F32, kind="ExternalInput").ap()
    bo_d = nc.dram_tensor("bo", [D], F32, kind="ExternalInput").ap()
    out_d = nc.dram_tensor("out", [S, D], BF16, kind="ExternalOutput").ap()

    with tile.TileContext(nc) as tc, ExitStack() as ctx:
        _body(ctx, tc, xt_d, x8_d, wqk_d, wv_d, wo_d, bqk_d, bv_d, bo_d, out_d)
    nc.compile()
    return nc


def _body(ctx, tc, xt_d, x8_d, wqk_d, wv_d, wo_d, bqk_d, bv_d, bo_d, out_d):
    nc = tc.nc
    const = ctx.enter_context(tc.tile_pool(name="const", bufs=1))
    ppool = ctx.enter_context(tc.tile_pool(name="ppool", bufs=8))
    spool = ctx.enter_context(tc.tile_pool(name="spool", bufs=4))
    opool = ctx.enter_context(tc.tile_pool(name="opool", bufs=4))
    ps_mm = ctx.enter_context(tc.tile_pool(name="ps_mm", bufs=3, space="PSUM"))
    ps_pj = ps_mm
    ps_z = ctx.enter_context(tc.tile_pool(name="ps_z", bufs=2, space="PSUM"))

    # --- engine warmups ----------------------------------------------------
    # DVE pays ~11us on its first real op; ACT pays a ~2.7us exp-table load.
    # Absorb both at t=0, concurrent with the input DMAs.
    warm = const.tile([1, 8], F32, tag="warm")
    nc.vector.memset(warm[:], 1.0)
    warmp = ps_z.tile([1, 8], F32, tag="z", name="warmp")
    nc.vector.tensor_copy(warmp[:], warm[:])
    warmb = const.tile([1, 8], BF16, tag="warmb")
    nc.vector.tensor_copy(warmb[:], warmp[:])  # preload DVE psum-read CAST path
    nc.scalar.activation(warm[:], warm[:], AF.Exp, scale=1.0)

    # --- constants ---------------------------------------------------------
    # trimask[r, c] = 1 if r <= c else 0 (keep k <= q in [k, q] layout)
    trimask = const.tile([P, P], BF16, tag="trimask")
    make_upper_triangular(nc, trimask[:], val=1.0, diag=True)

    # --- input DMAs --------------------------------------------------------
    xt = const.tile([P, DT, S], BF16, tag="xt")
    x8 = const.tile([P, DTP, 2, S], FP8, tag="x8")
    wqk = const.tile([P, NPAIR, 2, DTP, 2, P], FP8, tag="wqk8")
    wv_sb = const.tile([P, DT, N * H], BF16, tag="wv")
    wo_sb = const.tile([P, NPAIR, D], BF16, tag="wo")
    bqk_sb = const.tile([P, 2, NPAIR], F32, tag="bqk")
    bv_rep = const.tile([P, N * H], F32, tag="bvrep")
    bo_rep = const.tile([P, D], F32, tag="borep")

    # gpsimd SW queue: biases first (tiny; bq/bk needed at qk0 evac), then wv
    nc.gpsimd.dma_start(bqk_sb[:], bqk_d)
    nc.gpsimd.dma_start(
        bv_rep[:], bv_d[None, :].to_broadcast((P, N * H))
    )
    for dt in range(DT):
        nc.gpsimd.dma_start(wv_sb[:, dt, :], wv_d[:, dt, :])
    nc.gpsimd.dma_start(bo_rep[:], bo_d[None, :].to_broadcast((P, D)))

    # scalar HW queue: pair-0 weights + last x8 chunk lead, then odd xt
    # chunks, then the remaining weights
    nc.scalar.dma_start(wqk[:, 0], wqk_d[:, 0])
    nc.scalar.dma_start(x8[:, 2], x8_d[:, 2])
    for dt in (1, 3, 5):
        nc.scalar.dma_start(xt[:, dt, :], xt_d[:, dt, :])
    for pr in range(1, NPAIR):
        nc.scalar.dma_start(wqk[:, pr], wqk_d[:, pr])
    nc.scalar.dma_start(wo_sb[:], wo_d)

    # sync HW queue: first x8 chunks, then even xt chunks (out tiles ride
    # this queue later)
    for dtp in (0, 1):
        nc.sync.dma_start(x8[:, dtp], x8_d[:, dtp])
    for dt in (0, 2, 4):
        nc.sync.dma_start(xt[:, dt, :], xt_d[:, dt, :])

    # --- persistent attention tiles ---------------------------------------
    qT = const.tile([P, NPAIR, S], BF16, tag="qT")
    kT = const.tile([P, NPAIR, S], BF16, tag="kT")
    zT = const.tile([P, NPAIR, S], BF16, tag="zT")
    # V_aug: the 64-wide ones block makes the PV matmul replicate the softmax
    # denominators into PSUM partitions 64:128 -- broadcast for free.  Only
    # the ones region needs the memset; v_proj overwrites the value region.
    v_aug = const.tile([P, KT, N, 2 * H], BF16, tag="vaug")
    nc.vector.memset(v_aug[:, :, :, H : 2 * H], 1.0)

    def qk_proj_steps(pr):
        # Q^T (bank 0) and K^T (bank 1) of one 2-bank psum tile, as a list of
        # single-matmul closures so the pair-(pr) projection can be drip-fed
        # into pair-(pr-1)'s attention stream as PE filler work.  Q evacuates
        # on ACT (activation+bias), K on DVE, splitting the evac load.
        steps = []
        for sb_i in range(SB):
            box = {}

            def mk(dtp, half, sb_i=sb_i, box=box):
                def go():
                    if "t" not in box:
                        box["t"] = ps_pj.tile(
                            [P, 2 * QB], F32, tag="mm", name=f"pqk_{pr}_{sb_i}"
                        )
                    pqk = box["t"]
                    nc.tensor.matmul(
                        pqk[:, half * QB : (half + 1) * QB],
                        lhsT=wqk[:, pr, half, dtp],
                        rhs=x8[:, dtp, :, bass.ts(sb_i, QB)],
                        start=(dtp == 0),
                        stop=(dtp == DTP - 1),
                        perf_mode=DR,
                    )
                    if half == 1 and dtp == DTP - 1:
                        if ACT_QEVAC:
                            nc.scalar.activation(
                                qT[:, pr, bass.ts(sb_i, QB)], pqk[:, 0:QB],
                                AF.Identity, bias=bqk_sb[:, 0, pr : pr + 1],
                                scale=1.0 / WSCALE,
                            )
                        else:
                            nc.vector.tensor_scalar(
                                qT[:, pr, bass.ts(sb_i, QB)], pqk[:, 0:QB],
                                1.0 / WSCALE, bqk_sb[:, 0, pr : pr + 1],
                                ALU.mult, ALU.add,
                            )
                        nc.vector.tensor_scalar(
                            kT[:, pr, bass.ts(sb_i, QB)], pqk[:, QB : 2 * QB],
                            1.0 / WSCALE, bqk_sb[:, 1, pr : pr + 1],
                            ALU.mult, ALU.add,
                        )

                return go

            for half in range(2):
                for dtp in range(DTP):
                    steps.append(mk(dtp, half))
        return steps

    def qk_proj_sb(pr, sb_i):
        for s in qk_proj_steps(pr)[sb_i * 2 * DTP : (sb_i + 1) * 2 * DTP]:
            s()

    # --- V projection: dt-major, two 4-s-tile phases ----------------------
    # 8 concurrent PSUM accumulation groups per phase (3 ps_mm tiles hosting
    # 2 384-wide groups each + 2 ps_z tiles hosting 1), so the first matmuls
    # need only wv[:, 0] off the wire instead of the whole weight.
    def v_proj_phase(stiles, cb=None):
        # 2 ps_mm + 2 ps_z tiles per phase: one ps_mm ring slot stays free
        # so `cb` can interleave pair-0 S^T tiles (their exps run on the
        # otherwise-idle ACT during the V projection)
        n_groups = 2 * len(stiles)
        n_mm = (n_groups - 2 + 1) // 2
        tiles = [
            ps_mm.tile([P, 2 * QB], F32, tag="mm", name=f"pv{stiles[0]}_{i}")
            for i in range(n_mm)
        ]
        ztiles = [
            ps_z.tile([P, QB], F32, tag="z", name=f"pvz{stiles[0]}_{i}")
            for i in range(2)
        ]
        regions = []
        for g in range(n_groups):
            st, blk = stiles[g // 2], g % 2
            if g < 2 * n_mm:
                reg = tiles[g // 2][:, (g % 2) * QB : (g % 2) * QB + EB]
            else:
                reg = ztiles[g - 2 * n_mm][:, :EB]
            regions.append((reg, st, blk))
        for dt in range(DT):
            for reg, st, blk in regions:
                nc.tensor.matmul(
                    reg,
                    lhsT=xt[:, dt, bass.ts(st, P)],
                    rhs=wv_sb[:, dt, bass.ts(blk, EB)],
                    start=(dt == 0),
                    stop=(dt == DT - 1),
                )
            if cb is not None and dt == 3:
                cb()
        for reg, st, blk in regions:
            nc.vector.tensor_tensor(
                v_aug[:, st, bass.ts(blk, 6), 0:H],
                reg.rearrange("p (n h) -> p n h", h=H),
                bv_rep[:, bass.ts(blk, EB)].rearrange("p (n h) -> p n h", h=H),
                ALU.add,
            )

    def o_proj_steps(qts, alt_pool=False):
        # out[q, e] = z^T.T @ W_O + b_O, as single-matmul closures
        steps = []
        for qt in qts:
            for eb in range(D // EB):
                box = {}

                def mk(pr, qt=qt, eb=eb, box=box):
                    def go():
                        if "t" not in box:
                            # tail-only: alternate into the attention z-pool
                            # (free after the last pair) for deeper rotation
                            if alt_pool and (2 * qt + eb) % 2:
                                box["t"] = ps_z.tile(
                                    [P, QB], F32, tag="z", name=f"po_{qt}_{eb}"
                                )
                            else:
                                box["t"] = ps_mm.tile(
                                    [P, 2 * QB], F32, tag="mm", name=f"po_{qt}_{eb}"
                                )
                        po = box["t"]
                        nc.tensor.matmul(
                            po[:, :EB],
                            lhsT=zT[:, pr, bass.ts(qt, P)],
                            rhs=wo_sb[:, pr, bass.ts(eb, EB)],
                            start=(pr == 0),
                            stop=(pr == NPAIR - 1),
                        )
                        if pr == NPAIR - 1:
                            ot = opool.tile([P, EB], BF16, tag="ot")
                            nc.vector.tensor_tensor(
                                ot[:], po[:, :EB], bo_rep[:, bass.ts(eb, EB)],
                                ALU.add,
                            )
                            oeng = (nc.sync, nc.scalar, nc.gpsimd)[(2 * qt + eb) % 3]
                            oeng.dma_start(
                                out_d[bass.ts(qt, P), bass.ts(eb, EB)], ot[:]
                            )

                    return go

                for pr in range(NPAIR):
                    steps.append(mk(pr))
        return steps

    o_first = o_proj_steps(range(4))  # q-tiles 0-3: fills attn(last, j=1)
    o_idx = [0]

    # --- attention block machinery ----------------------------------------

    class Block:
        def __init__(self, pr, j):
            self.pr, self.j = pr, j
            self.n_kt = 4 * (j + 1)
            self.pz = None
            self.pts = {}
            self.normed = False

        def alloc_pz(self):
            if self.pz is None:
                self.pz = [
                    ps_z.tile([P, QB], F32, tag="z", name=f"z_{self.pr}_{self.j}_{h}")
                    for h in range(2)
                ]

        def emit_st(self, i):
            # S^T for both halves into one 2-bank tile; exp; mask
            pr, j = self.pr, self.j
            q_off = max(0, (i - 4 * j) * P)
            ps = ps_mm.tile([P, 2 * QB], F32, tag="mm", name=f"s_{pr}_{j}_{i}")
            for half in range(2):
                lo, hi = 64 * half, 64 * half + 64
                nc.tensor.matmul(
                    ps[:, half * QB + q_off : (half + 1) * QB],
                    lhsT=kT[lo:hi, pr, bass.ts(i, P)],
                    rhs=qT[lo:hi, pr, j * QB + q_off : (j + 1) * QB],
                    start=True,
                    stop=True,
                )
            pT = ppool.tile([P, 2, QB], BF16, tag="pT")
            ps3 = ps.rearrange("p (h q) -> p h q", h=2)
            nc.scalar.activation(
                pT[:, :, q_off:], ps3[:, :, q_off:], AF.Exp, scale=0.125
            )
            if i >= 4 * j:  # diagonal tile: triangular mask, both halves
                mask_eng = nc.vector if DVE_MASK else nc.gpsimd
                mask_eng.tensor_tensor(
                    pT[:, :, q_off : q_off + P],
                    pT[:, :, q_off : q_off + P],
                    trimask[:, None, :].to_broadcast((P, 2, P)),
                    ALU.mult,
                )
            self.pts[i] = pT

        def emit_pv(self, i):
            pr, j = self.pr, self.j
            self.alloc_pz()
            q_off = max(0, (i - 4 * j) * P)
            for half in range(2):
                n = 2 * pr + half
                nc.tensor.matmul(
                    self.pz[half][:, q_off:],
                    lhsT=v_aug[:, i, n, :],
                    rhs=self.pts[i][:, half, q_off:],
                    start=(i == 0),
                    stop=(i == self.n_kt - 1),
                )
            del self.pts[i]

        def emit_norm(self):
            # normalize z and store z^T (PSUM rows 64:128 hold the
            # denominators replicated by the ones block)
            pr, j = self.pr, self.j
            for half in range(2):
                lo, hi = 64 * half, 64 * half + 64
                if PSUM_RECIP:
                    den = self.pz[half][H : 2 * H, :]
                else:
                    sm = spool.tile([64, QB], F32, tag="sm")
                    nc.vector.tensor_copy(sm[:], self.pz[half][H : 2 * H, :])
                    den = sm[:]
                rc = spool.tile([64, QB], F32, tag="rc")
                nc.vector.reciprocal_approx_fast(rc[:], den)
                nc.vector.tensor_mul(
                    zT[lo:hi, pr, bass.ts(j, QB)], self.pz[half][0:H, :], rc[:]
                )
            self.normed = True

    # --- emission: qk0 | v phases (pair-0 S^T/exp interleaved) | pairs ----
    # Pair-0 j=0's S^T tiles ride the free ps_mm ring slot inside the V
    # projection: their exps run on the otherwise-idle ACT, so pair 0
    # opens with its P^T tiles already computed.
    qk_proj_sb(0, 0)
    qk_proj_sb(0, 1)
    b00 = Block(0, 0)
    st_cnt = [0]

    def _stcb():
        if st_cnt[0] < b00.n_kt:
            b00.emit_st(st_cnt[0])
            st_cnt[0] += 1

    v_proj_phase((0, 1, 2), _stcb)
    v_proj_phase((3, 4, 5), _stcb)
    v_proj_phase((6, 7), _stcb)
    _stcb()  # remaining pair-0 j=0 tile

    # v5 emission: pair-major, per-block loop, drip-fed filler
    for pr in range(NPAIR):
        last = pr + 1 >= NPAIR
        fill = qk_proj_steps(pr + 1) if not last else None
        fill_i = [0]

        def emit_fill(k=1):
            for _ in range(k):
                if fill is not None:
                    if fill_i[0] < len(fill):
                        fill[fill_i[0]]()
                        fill_i[0] += 1
                elif cur_j[0] == 1 and o_idx[0] < len(o_first):
                    o_first[o_idx[0]]()
                    o_idx[0] += 1

        cur_j = [0]
        for j in range(SB):
            cur_j[0] = j
            b = b00 if (pr == 0 and j == 0) else Block(pr, j)
            n_kt = b.n_kt
            # the final block runs a shorter PV lookahead so its drain ends
            # (and the fully-exposed last normalize starts) earlier
            final = pr == NPAIR - 1 and j == SB - 1
            la = 3 if final else LOOKAHEAD
            for i in range(n_kt):
                if not (b is b00 and i < st_cnt[0]):
                    b.emit_st(i)
                emit_fill(2 if i < 2 else 1)
                if i >= la:
                    b.emit_pv(i - la)
                    emit_fill()
            for i in range(max(0, n_kt - la), n_kt):
                b.emit_pv(i)
                if not final:
                    emit_fill()
            b.emit_norm()

    # --- output projection leftovers -----------------------------------
    for s in o_first[o_idx[0] :]:
        s()
    for s in o_proj_steps(range(4, KT), alt_pool=True):
        s()


_CACHE = {}


def get_nc():
    if "nc" not in _CACHE:
        _CACHE["nc"] = _build_nc()
    return _CACHE["nc"]


def _prep_shared(W_Q, W_K, W_V, W_O, b_Q, b_K, b_V, b_O):
    W_Q = np.asarray(W_Q, np.float32)
    W_K = np.asarray(W_K, np.float32)
    W_V = np.asarray(W_V, np.float32)
    W_O = np.asarray(W_O, np.float32)
    # wqk8 [dp, pr, half, dtp, j, a*64+h], prescaled by WSCALE
    def qk_img(W):
        return (W * WSCALE).reshape(NPAIR, 2, DTP, 2, P, H).transpose(4, 0, 2, 3, 1, 5)
    wqk = np.stack([qk_img(W_Q), qk_img(W_K)], axis=2).reshape(
        P, NPAIR, 2, DTP, 2, P
    ).astype(NPFP8)
    wv = np.ascontiguousarray(
        W_V.reshape(N, DT, P, H).transpose(2, 1, 0, 3).reshape(P, DT, N * H)
    ).astype(NPBF16)
    wo = np.ascontiguousarray(
        W_O.reshape(NPAIR, 2, H, D).transpose(1, 2, 0, 3).reshape(P, NPAIR, D)
    ).astype(NPBF16)
    def b_img(b):
        return np.asarray(b, np.float32).reshape(NPAIR, 2, H).transpose(1, 2, 0).reshape(P, NPAIR)
    bqk = np.ascontiguousarray(
        np.stack([b_img(b_Q), b_img(b_K)], axis=1)
    ).astype(np.float32)
    return {
        "wqk8": np.ascontiguousarray(wqk),
        "wv": wv,
        "wo": wo,
        "bqk": bqk,
        "bv": np.ascontiguousarray(np.asarray(b_V, np.float32).reshape(N * H)),
        "bo": np.ascontiguousarray(np.asarray(b_O, np.float32)),
    }


def _prep_x(xb):
    # xt: [1024, 768] f32 -> [128, 6, 1024] bf16 (partition = d%128)
    # x8: same data as fp8, d-tiles pair-interleaved for DoubleRow
    xT = xb.T.reshape(DT, P, S)
    xt = np.ascontiguousarray(xT.transpose(1, 0, 2)).astype(NPBF16)
    x8 = np.ascontiguousarray(
        xT.reshape(DTP, 2, P, S).transpose(2, 0, 1, 3)
    ).astype(NPFP8)
    return {"xt": xt, "x8": x8}


def kernel(normalized_resid_pre, W_Q, W_K, W_V, W_O, b_Q, b_K, b_V, b_O, **kw):
    x = np.asarray(normalized_resid_pre, dtype=np.float32)
    shared = _prep_shared(W_Q, W_K, W_V, W_O, b_Q, b_K, b_V, b_O)
    in_maps = [dict(shared, **_prep_x(x[b])) for b in range(B)]
    nc = get_nc()
    res = run_bass_kernel_spmd(nc, in_maps, core_ids=list(range(N_CORES)))
    return np.stack(
        [np.asarray(res.results[b]["out"], np.float32) for b in range(B)], axis=0
    )


# revision 48
# speedup vs baseline: 1.1819x; 1.0157x over previous
"""Trainium2 Bass kernel for nn_Attention_18726057410905.

Multi-head causal attention: B=8, S=1024, D=768, N=12 heads, H=64.
Sharding: data-parallel over batch -- core b computes batch element b.
No collectives.

v2: all operands are pre-laid-out on the HOST into their exact SBUF
images and uploaded as bf16 (halves HBM traffic, kills the on-chip
x-transposes / casts / staging DMAs of v1):
  xt   [128, 6, 1024]  x^T, partition = d%128, free = (d//128, s)
  wqk  [128, 6, 2, 6, 128]  [dp, pair, q/k, dt, 2*64 packed heads]
  wv   [128, 6, 768]   [dp, dt, (n h)]
  wo   [128, 6, 768]   [2 packed heads * 64, pair, e]
  bqk  [128, 2, 6]     packed-head-major Q/K biases
  bv/bo [768]          flat; broadcast-DMA'd to [128, .] on chip

Per-core dataflow (matmul inputs bf16, fp32 PSUM accumulation):
  Q^T,K^T [2*64h, s] per head-pair (W stationary, xt moving)
  V_aug [s, n, 128]  natural layout + 64-wide ones block (cols 64:128)
  S^T   [k-tile 128, 2 halves x 512q] -- one 2-bank PSUM tile per k-tile,
        2 heads row-packed on the PE (K=64 contraction, concurrent via
        row tiling)
  P^T   = exp(S^T/8) via one ACT activation per k-tile; triangular mask
          (DVE) on diagonal tiles only; fully-masked tiles never computed
  z_aug^T [128, q] = sum_k V_aug.T @ P^T; rows 64:128 hold the softmax
        denominators replicated by the ones block (broadcast for free)
  z^T normalized with reciprocal_approx_fast + multiply
  out   [q, e] = z^T.T @ W_O + b_O

DMA plan: xt split across the sync and scalar HW-DGE queues per d-tile;
wqk pair 0 leads the scalar queue, pairs 1-5 + wo follow the xt chunks;
wv + biases ride the gpsimd SW-DGE queue.  Out tiles go back on sync.

Pipelining: PV matmuls trail S^T/exp by LOOKAHEAD k-tiles; the next pair's
Q/K projection matmuls (and, for the last pair, the output projection) are
drip-fed into the attention stream as PE filler so the in-order PE never
idles on the ACT exp stream.
"""

from contextlib import ExitStack

import numpy as np
import ml_dtypes

import concourse.bass as bass
import concourse.tile as tile
from concourse import bacc, mybir
from concourse.bass_utils import run_bass_kernel_spmd
from concourse.masks import make_upper_triangular

B, S, D, N, H = 8, 1024, 768, 12, 64
P = 128
N_CORES = 8
DT = D // P          # 6 d-tiles
NPAIR = N // 2       # 6 head pairs
QB = 512             # q-block width
SB = S // QB         # 2 q/s blocks
KT = S // P          # 8 k/s tiles
EB = 384             # e-block width for the output projection
LOOKAHEAD = 6        # k-tiles of PV deferral (keeps PE fed while ACT exps)
BF16 = mybir.dt.bfloat16
F32 = mybir.dt.float32
AF = mybir.ActivationFunctionType
ALU = mybir.AluOpType
NPBF16 = ml_dtypes.bfloat16
FP8 = mybir.dt.float8e4
NPFP8 = mybir.dt.np(FP8)
DR = mybir.MatmulPerfMode.DoubleRow
DTP = DT // 2        # 3 doubled d-tiles for the fp8 DoubleRow QK projection
WSCALE = 64.0        # fp8 weight prescale (keeps W_Q/W_K out of e4m3 subnormals)

# mechanism toggles
ACT_QEVAC = False    # Q^T evac on ACT via activation Identity+bias
PSUM_RECIP = False   # reciprocal_approx_fast reads denominators from PSUM
DVE_MASK = False     # triangular mask on DVE instead of gpsimd


def _build_nc():
    nc = bacc.Bacc(
        "TRN2", target_bir_lowering=False, debug=False, num_devices=N_CORES
    )
    xt_d = nc.dram_tensor("xt", [P, DT, S], BF16, kind="ExternalInput").ap()
    x8_d = nc.dram_tensor("x8", [P, DTP, 2, S], FP8, kind="ExternalInput").ap()
    wqk_d = nc.dram_tensor("wqk8", [P, NPAIR, 2, DTP, 2, P], FP8, kind="ExternalInput").ap()
    wv_d = nc.dram_tensor("wv", [P, DT, N * H], BF16, kind="ExternalInput").ap()
    wo_d = nc.dram_tensor("wo", [P, NPAIR, D], BF16, kind="ExternalInput").ap()
    bqk_d = nc.dram_tensor("bqk", [P, 2, NPAIR], F32, kind="ExternalInput").ap()
    bv_d = nc.dram_tensor("bv", [N * H], F32, kind="ExternalInput").ap()
    bo_d = nc.dram_tensor("bo", [D], F32, kind="ExternalInput").ap()
    out_d = nc.dram_tensor("out", [S, D], BF16, kind="ExternalOutput").ap()

    with tile.TileContext(nc) as tc, ExitStack() as ctx:
        _body(ctx, tc, xt_d, x8_d, wqk_d, wv_d, wo_d, bqk_d, bv_d, bo_d, out_d)
    nc.compile()
    return nc


def _body(ctx, tc, xt_d, x8_d, wqk_d, wv_d, wo_d, bqk_d, bv_d, bo_d, out_d):
    nc = tc.nc
    const = ctx.enter_context(tc.tile_pool(name="const", bufs=1))
    ppool = ctx.enter_context(tc.tile_pool(name="ppool", bufs=8))
    spool = ctx.enter_context(tc.tile_pool(name="spool", bufs=4))
    opool = ctx.enter_context(tc.tile_pool(name="opool", bufs=4))
    ps_mm = ctx.enter_context(tc.tile_pool(name="ps_mm", bufs=3, space="PSUM"))
    ps_pj = ps_mm
    ps_z = ctx.enter_context(tc.tile_pool(name="ps_z", bufs=2, space="PSUM"))

    # --- engine warmups ----------------------------------------------------
    # DVE pays ~11us on its first real op; ACT pays a ~2.7us exp-table load.
    # Absorb both at t=0, concurrent with the input DMAs.
    warm = const.tile([1, 8], F32, tag="warm")
    nc.vector.memset(warm[:], 1.0)
    warmp = ps_z.tile([1, 8], F32, tag="z", name="warmp")
    nc.vector.tensor_copy(warmp[:], warm[:])
    warmb = const.tile([1, 8], BF16, tag="warmb")
    nc.vector.tensor_copy(warmb[:], warmp[:])  # preload DVE psum-read CAST path
    nc.scalar.activation(warm[:], warm[:], AF.Exp, scale=1.0)

    # --- constants ---------------------------------------------------------
    # trimask[r, c] = 1 if r <= c else 0 (keep k <= q in [k, q] layout)
    trimask = const.tile([P, P], BF16, tag="trimask")
    make_upper_triangular(nc, trimask[:], val=1.0, diag=True)

    # --- input DMAs --------------------------------------------------------
    xt = const.tile([P, DT, S], BF16, tag="xt")
    x8 = const.tile([P, DTP, 2, S], FP8, tag="x8")
    wqk = const.tile([P, NPAIR, 2, DTP, 2, P], FP8, tag="wqk8")
    wv_sb = const.tile([P, DT, N * H], BF16, tag="wv")
    wo_sb = const.tile([P, NPAIR, D], BF16, tag="wo")
    bqk_sb = const.tile([P, 2, NPAIR], F32, tag="bqk")
    bv_rep = const.tile([P, N * H], F32, tag="bvrep")
    bo_rep = const.tile([P, D], F32, tag="borep")

    # gpsimd SW queue: biases first (tiny; bq/bk needed at qk0 evac), then wv
    nc.gpsimd.dma_start(bqk_sb[:], bqk_d)
    nc.gpsimd.dma_start(
        bv_rep[:], bv_d[None, :].to_broadcast((P, N * H))
    )
    for dt in range(DT):
        nc.gpsimd.dma_start(wv_sb[:, dt, :], wv_d[:, dt, :])
    nc.gpsimd.dma_start(bo_rep[:], bo_d[None, :].to_broadcast((P, D)))

    # scalar HW queue: pair-0 weights + last x8 chunk lead, then odd xt
    # chunks, then the remaining weights
    nc.scalar.dma_start(wqk[:, 0], wqk_d[:, 0])
    nc.scalar.dma_start(x8[:, 2], x8_d[:, 2])
    for dt in (1, 3, 5):
        nc.scalar.dma_start(xt[:, dt, :], xt_d[:, dt, :])
    for pr in range(1, NPAIR):
        nc.scalar.dma_start(wqk[:, pr], wqk_d[:, pr])
    nc.scalar.dma_start(wo_sb[:], wo_d)

    # sync HW queue: first x8 chunks, then even xt chunks (out tiles ride
    # this queue later)
    for dtp in (0, 1):
        nc.sync.dma_start(x8[:, dtp], x8_d[:, dtp])
    for dt in (0, 2, 4):
        nc.sync.dma_start(xt[:, dt, :], xt_d[:, dt, :])

    # --- persistent attention tiles ---------------------------------------
    qT = const.tile([P, NPAIR, S], BF16, tag="qT")
    kT = const.tile([P, NPAIR, S], BF16, tag="kT")
    zT = const.tile([P, NPAIR, S], BF16, tag="zT")
    # V_aug: the 64-wide ones block makes the PV matmul replicate the softmax
    # denominators into PSUM partitions 64:128 -- broadcast for free.  Only
    # the ones region needs the memset; v_proj overwrites the value region.
    v_aug = const.tile([P, KT, N, 2 * H], BF16, tag="vaug")
    nc.vector.memset(v_aug[:, :, :, H : 2 * H], 1.0)

    def qk_proj_steps(pr):
        # Q^T (bank 0) and K^T (bank 1) of one 2-bank psum tile, as a list of
        # single-matmul closures so the pair-(pr) projection can be drip-fed
        # into pair-(pr-1)'s attention stream as PE filler work.  Q evacuates
        # on ACT (activation+bias), K on DVE, splitting the evac load.
        steps = []
        for sb_i in range(SB):
            box = {}

            def mk(dtp, half, sb_i=sb_i, box=box):
                def go():
                    if "t" not in box:
                        box["t"] = ps_pj.tile(
                            [P, 2 * QB], F32, tag="mm", name=f"pqk_{pr}_{sb_i}"
                        )
                    pqk = box["t"]
                    nc.tensor.matmul(
                        pqk[:, half * QB : (half + 1) * QB],
                        lhsT=wqk[:, pr, half, dtp],
                        rhs=x8[:, dtp, :, bass.ts(sb_i, QB)],
                        start=(dtp == 0),
                        stop=(dtp == DTP - 1),
                        perf_mode=DR,
                    )
                    if half == 1 and dtp == DTP - 1:
                        if ACT_QEVAC:
                            nc.scalar.activation(
                                qT[:, pr, bass.ts(sb_i, QB)], pqk[:, 0:QB],
                                AF.Identity, bias=bqk_sb[:, 0, pr : pr + 1],
                                scale=1.0 / WSCALE,
                            )
                        else:
                            nc.vector.tensor_scalar(
                                qT[:, pr, bass.ts(sb_i, QB)], pqk[:, 0:QB],
                                1.0 / WSCALE, bqk_sb[:, 0, pr : pr + 1],
                                ALU.mult, ALU.add,
                            )
                        nc.vector.tensor_scalar(
                            kT[:, pr, bass.ts(sb_i, QB)], pqk[:, QB : 2 * QB],
                            1.0 / WSCALE, bqk_sb[:, 1, pr : pr + 1],
                            ALU.mult, ALU.add,
                        )

                return go

            for half in range(2):
                for dtp in range(DTP):
                    steps.append(mk(dtp, half))
        return steps

    def qk_proj_sb(pr, sb_i):
        for s in qk_proj_steps(pr)[sb_i * 2 * DTP : (sb_i + 1) * 2 * DTP]:
            s()

    # --- V projection: dt-major, two 4-s-tile phases ----------------------
    # 8 concurrent PSUM accumulation groups per phase (3 ps_mm tiles hosting
    # 2 384-wide groups each + 2 ps_z tiles hosting 1), so the first matmuls
    # need only wv[:, 0] off the wire instead of the whole weight.
    def v_proj_phase(stiles, cb=None):
        # 2 ps_mm + 2 ps_z tiles per phase: one ps_mm ring slot stays free
        # so `cb` can interleave pair-0 S^T tiles (their exps run on the
        # otherwise-idle ACT during the V projection)
        n_groups = 2 * len(stiles)
        n_mm = (n_groups - 2 + 1) // 2
        tiles = [
            ps_mm.tile([P, 2 * QB], F32, tag="mm", name=f"pv{stiles[0]}_{i}")
            for i in range(n_mm)
        ]
        ztiles = [
            ps_z.tile([P, QB], F32, tag="z", name=f"pvz{stiles[0]}_{i}")
            for i in range(2)
        ]
        regions = []
        for g in range(n_groups):
            st, blk = stiles[g // 2], g % 2
            if g < 2 * n_mm:
                reg = tiles[g // 2][:, (g % 2) * QB : (g % 2) * QB + EB]
            else:
                reg = ztiles[g - 2 * n_mm][:, :EB]
            regions.append((reg, st, blk))
        for dt in range(DT):
            for reg, st, blk in regions:
                nc.tensor.matmul(
                    reg,
                    lhsT=xt[:, dt, bass.ts(st, P)],
                    rhs=wv_sb[:, dt, bass.ts(blk, EB)],
                    start=(dt == 0),
                    stop=(dt == DT - 1),
                )
            if cb is not None and dt == 3:
                cb()
        for reg, st, blk in regions:
            nc.vector.tensor_tensor(
                v_aug[:, st, bass.ts(blk, 6), 0:H],
                reg.rearrange("p (n h) -> p n h", h=H),
                bv_rep[:, bass.ts(blk, EB)].rearrange("p (n h) -> p n h", h=H),
                ALU.add,
            )

    def o_proj_steps(qts, alt_pool=False):
        # out[q, e] = z^T.T @ W_O + b_O, as single-matmul closures
        steps = []
        for qt in qts:
            for eb in range(D // EB):
                box = {}

                def mk(pr, qt=qt, eb=eb, box=box):
                    def go():
                        if "t" not in box:
                            # tail-only: alternate into the attention z-pool
                            # (free after the last pair) for deeper rotation
                            if alt_pool and (2 * qt + eb) % 2:
                                box["t"] = ps_z.tile(
                                    [P, QB], F32, tag="z", name=f"po_{qt}_{eb}"
                                )
                            else:
                                box["t"] = ps_mm.tile(
                                    [P, 2 * QB], F32, tag="mm", name=f"po_{qt}_{eb}"
                                )
                        po = box["t"]
                        nc.tensor.matmul(
                            po[:, :EB],
                            lhsT=zT[:, pr, bass.ts(qt, P)],
                            rhs=wo_sb[:, pr, bass.ts(eb, EB)],
                            start=(pr == 0),
                            stop=(pr == NPAIR - 1),
                        )
                        if pr == NPAIR - 1:
                            ot = opool.tile([P, EB], BF16, tag="ot")
                            nc.vector.tensor_tensor(
                                ot[:], po[:, :EB], bo_rep[:, bass.ts(eb, EB)],
                                ALU.add,
                            )
                            oeng = (nc.sync, nc.scalar, nc.gpsimd)[(2 * qt + eb) % 3]
                            oeng.dma_start(
                                out_d[bass.ts(qt, P), bass.ts(eb, EB)], ot[:]
                            )

                    return go

                for pr in range(NPAIR):
                    steps.append(mk(pr))
        return steps

    o_first = o_proj_steps(range(4))  # q-tiles 0-3: fills attn(last, j=1)
    o_idx = [0]

    # --- attention block machinery ----------------------------------------

    class Block:
        def __init__(self, pr, j):
            self.pr, self.j = pr, j
            self.n_kt = 4 * (j + 1)
            self.pz = None
            self.pts = {}
            self.normed = False

        def alloc_pz(self):
            if self.pz is None:
                self.pz = [
                    ps_z.tile([P, QB], F32, tag="z", name=f"z_{self.pr}_{self.j}_{h}")
                    for h in range(2)
                ]

        def emit_st(self, i):
            # S^T for both halves into one 2-bank tile; exp; mask
            pr, j = self.pr, self.j
            q_off = max(0, (i - 4 * j) * P)
            ps = ps_mm.tile([P, 2 * QB], F32, tag="mm", name=f"s_{pr}_{j}_{i}")
            for half in range(2):
                lo, hi = 64 * half, 64 * half + 64
                nc.tensor.matmul(
                    ps[:, half * QB + q_off : (half + 1) * QB],
                    lhsT=kT[lo:hi, pr, bass.ts(i, P)],
                    rhs=qT[lo:hi, pr, j * QB + q_off : (j + 1) * QB],
                    start=True,
                    stop=True,
                )
            pT = ppool.tile([P, 2, QB], BF16, tag="pT")
            ps3 = ps.rearrange("p (h q) -> p h q", h=2)
            nc.scalar.activation(
                pT[:, :, q_off:], ps3[:, :, q_off:], AF.Exp, scale=0.125
            )
            if i >= 4 * j:  # diagonal tile: triangular mask, both halves
                mask_eng = nc.vector if DVE_MASK else nc.gpsimd
                mask_eng.tensor_tensor(
                    pT[:, :, q_off : q_off + P],
                    pT[:, :, q_off : q_off + P],
                    trimask[:, None, :].to_broadcast((P, 2, P)),
                    ALU.mult,
                )
            self.pts[i] = pT

        def emit_pv(self, i):
            pr, j = self.pr, self.j
            self.alloc_pz()
            q_off = max(0, (i - 4 * j) * P)
            for half in range(2):
                n = 2 * pr + half
                nc.tensor.matmul(
                    self.pz[half][:, q_off:],
                    lhsT=v_aug[:, i, n, :],
                    rhs=self.pts[i][:, half, q_off:],
                    start=(i == 0),
                    stop=(i == self.n_kt - 1),
                )
            del self.pts[i]

        def emit_norm(self, merged=False):
            # normalize z and store z^T (PSUM rows 64:128 hold the
            # denominators replicated by the ones block).  merged=True packs
            # both halves into one reciprocal (5 DVE ops instead of 6) --
            # used for the final block, whose norm chain is fully exposed.
            pr, j = self.pr, self.j
            if merged:
                sm = spool.tile([P, QB], F32, tag="smm", name=f"smm_{pr}_{j}")
                for half in range(2):
                    nc.vector.tensor_copy(
                        sm[64 * half : 64 * half + 64, :],
                        self.pz[half][H : 2 * H, :],
                    )
                rc = spool.tile([P, QB], F32, tag="rcm", name=f"rcm_{pr}_{j}")
                nc.vector.reciprocal_approx_fast(rc[:], sm[:])
                for half in range(2):
                    lo, hi = 64 * half, 64 * half + 64
                    nc.vector.tensor_mul(
                        zT[lo:hi, pr, bass.ts(j, QB)],
                        self.pz[half][0:H, :], rc[lo:hi, :],
                    )
                self.normed = True
                return
            for half in range(2):
                lo, hi = 64 * half, 64 * half + 64
                if PSUM_RECIP:
                    den = self.pz[half][H : 2 * H, :]
                else:
                    sm = spool.tile([64, QB], F32, tag="sm")
                    nc.vector.tensor_copy(sm[:], self.pz[half][H : 2 * H, :])
                    den = sm[:]
                rc = spool.tile([64, QB], F32, tag="rc")
                nc.vector.reciprocal_approx_fast(rc[:], den)
                nc.vector.tensor_mul(
                    zT[lo:hi, pr, bass.ts(j, QB)], self.pz[half][0:H, :], rc[:]
                )
            self.normed = True

    # --- emission: qk0 | v phases (pair-0 S^T/exp interleaved) | pairs ----
    # Pair-0 j=0's S^T tiles ride the free ps_mm ring slot inside the V
    # projection: their exps run on the otherwise-idle ACT, so pair 0
    # opens with its P^T tiles already computed.
    qk_proj_sb(0, 0)
    qk_proj_sb(0, 1)
    b00 = Block(0, 0)
    st_cnt = [0]

    def _stcb():
        if st_cnt[0] < b00.n_kt:
            b00.emit_st(st_cnt[0])
            st_cnt[0] += 1

    v_proj_phase((0, 1, 2), _stcb)
    v_proj_phase((3, 4, 5), _stcb)
    v_proj_phase((6, 7), _stcb)
    _stcb()  # remaining pair-0 j=0 tile

    # v5 emission: pair-major, per-block loop, drip-fed filler
    for pr in range(NPAIR):
        last = pr + 1 >= NPAIR
        fill = qk_proj_steps(pr + 1) if not last else None
        fill_i = [0]

        def emit_fill(k=1):
            for _ in range(k):
                if fill is not None:
                    if fill_i[0] < len(fill):
                        fill[fill_i[0]]()
                        fill_i[0] += 1
                elif cur_j[0] == 1 and o_idx[0] < len(o_first):
                    o_first[o_idx[0]]()
                    o_idx[0] += 1

        cur_j = [0]
        for j in range(SB):
            cur_j[0] = j
            b = b00 if (pr == 0 and j == 0) else Block(pr, j)
            n_kt = b.n_kt
            # the final block runs a shorter PV lookahead so its drain ends
            # (and the fully-exposed last normalize starts) earlier
            final = pr == NPAIR - 1 and j == SB - 1
            la = 3 if final else LOOKAHEAD
            for i in range(n_kt):
                if not (b is b00 and i < st_cnt[0]):
                    b.emit_st(i)
                emit_fill(2 if i < 2 else 1)
                if i >= la:
                    b.emit_pv(i - la)
                    emit_fill()
            for i in range(max(0, n_kt - la), n_kt):
                b.emit_pv(i)
                if not final:
                    emit_fill()
            b.emit_norm(merged=final)

    # --- output projection leftovers -----------------------------------
    for s in o_first[o_idx[0] :]:
        s()
    for s in o_proj_steps(range(4, KT), alt_pool=True):
        s()


_CACHE = {}


def get_nc():
    if "nc" not in _CACHE:
        _CACHE["nc"] = _build_nc()
    return _CACHE["nc"]


def _prep_shared(W_Q, W_K, W_V, W_O, b_Q, b_K, b_V, b_O):
    W_Q = np.asarray(W_Q, np.float32)
    W_K = np.asarray(W_K, np.float32)
    W_V = np.asarray(W_V, np.float32)
    W_O = np.asarray(W_O, np.float32)
    # wqk8 [dp, pr, half, dtp, j, a*64+h], prescaled by WSCALE
    def qk_img(W):
        return (W * WSCALE).reshape(NPAIR, 2, DTP, 2, P, H).transpose(4, 0, 2, 3, 1, 5)
    wqk = np.stack([qk_img(W_Q), qk_img(W_K)], axis=2).reshape(
        P, NPAIR, 2, DTP, 2, P
    ).astype(NPFP8)
    wv = np.ascontiguousarray(
        W_V.reshape(N, DT, P, H).transpose(2, 1, 0, 3).reshape(P, DT, N * H)
    ).astype(NPBF16)
    wo = np.ascontiguousarray(
        W_O.reshape(NPAIR, 2, H, D).transpose(1, 2, 0, 3).reshape(P, NPAIR, D)
    ).astype(NPBF16)
    def b_img(b):
        return np.asarray(b, np.float32).reshape(NPAIR, 2, H).transpose(1, 2, 0).reshape(P, NPAIR)
    bqk = np.ascontiguousarray(
        np.stack([b_img(b_Q), b_img(b_K)], axis=1)
    ).astype(np.float32)
    return {
        "wqk8": np.ascontiguousarray(wqk),
        "wv": wv,
        "wo": wo,
        "bqk": bqk,
        "bv": np.ascontiguousarray(np.asarray(b_V, np.float32).reshape(N * H)),
        "bo": np.ascontiguousarray(np.asarray(b_O, np.float32)),
    }


def _prep_x(xb):
    # xt: [1024, 768] f32 -> [128, 6, 1024] bf16 (partition = d%128)
    # x8: same data as fp8, d-tiles pair-interleaved for DoubleRow
    xT = xb.T.reshape(DT, P, S)
    xt = np.ascontiguousarray(xT.transpose(1, 0, 2)).astype(NPBF16)
    x8 = np.ascontiguousarray(
        xT.reshape(DTP, 2, P, S).transpose(2, 0, 1, 3)
    ).astype(NPFP8)
    return {"xt": xt, "x8": x8}


def kernel(normalized_resid_pre, W_Q, W_K, W_V, W_O, b_Q, b_K, b_V, b_O, **kw):
    x = np.asarray(normalized_resid_pre, dtype=np.float32)
    shared = _prep_shared(W_Q, W_K, W_V, W_O, b_Q, b_K, b_V, b_O)
    in_maps = [dict(shared, **_prep_x(x[b])) for b in range(B)]
    nc = get_nc()
    res = run_bass_kernel_spmd(nc, in_maps, core_ids=list(range(N_CORES)))
    return np.stack(
        [np.asarray(res.results[b]["out"], np.float32) for b in range(B)], axis=0
    )


# revision 49
# speedup vs baseline: 1.1874x; 1.0046x over previous
"""Trainium2 Bass kernel for nn_Attention_18726057410905.

Multi-head causal attention: B=8, S=1024, D=768, N=12 heads, H=64.
Sharding: data-parallel over batch -- core b computes batch element b.
No collectives.

v2: all operands are pre-laid-out on the HOST into their exact SBUF
images and uploaded as bf16 (halves HBM traffic, kills the on-chip
x-transposes / casts / staging DMAs of v1):
  xt   [128, 6, 1024]  x^T, partition = d%128, free = (d//128, s)
  wqk  [128, 6, 2, 6, 128]  [dp, pair, q/k, dt, 2*64 packed heads]
  wv   [128, 6, 768]   [dp, dt, (n h)]
  wo   [128, 6, 768]   [2 packed heads * 64, pair, e]
  bqk  [128, 2, 6]     packed-head-major Q/K biases
  bv/bo [768]          flat; broadcast-DMA'd to [128, .] on chip

Per-core dataflow (matmul inputs bf16, fp32 PSUM accumulation):
  Q^T,K^T [2*64h, s] per head-pair (W stationary, xt moving)
  V_aug [s, n, 128]  natural layout + 64-wide ones block (cols 64:128)
  S^T   [k-tile 128, 2 halves x 512q] -- one 2-bank PSUM tile per k-tile,
        2 heads row-packed on the PE (K=64 contraction, concurrent via
        row tiling)
  P^T   = exp(S^T/8) via one ACT activation per k-tile; triangular mask
          (DVE) on diagonal tiles only; fully-masked tiles never computed
  z_aug^T [128, q] = sum_k V_aug.T @ P^T; rows 64:128 hold the softmax
        denominators replicated by the ones block (broadcast for free)
  z^T normalized with reciprocal_approx_fast + multiply
  out   [q, e] = z^T.T @ W_O + b_O

DMA plan: xt split across the sync and scalar HW-DGE queues per d-tile;
wqk pair 0 leads the scalar queue, pairs 1-5 + wo follow the xt chunks;
wv + biases ride the gpsimd SW-DGE queue.  Out tiles go back on sync.

Pipelining: PV matmuls trail S^T/exp by LOOKAHEAD k-tiles; the next pair's
Q/K projection matmuls (and, for the last pair, the output projection) are
drip-fed into the attention stream as PE filler so the in-order PE never
idles on the ACT exp stream.
"""

from contextlib import ExitStack

import numpy as np
import ml_dtypes

import concourse.bass as bass
import concourse.tile as tile
from concourse import bacc, mybir
from concourse.bass_utils import run_bass_kernel_spmd
from concourse.masks import make_upper_triangular

B, S, D, N, H = 8, 1024, 768, 12, 64
P = 128
N_CORES = 8
DT = D // P          # 6 d-tiles
NPAIR = N // 2       # 6 head pairs
QB = 512             # q-block width
SB = S // QB         # 2 q/s blocks
KT = S // P          # 8 k/s tiles
EB = 384             # e-block width for the output projection
LOOKAHEAD = 6        # k-tiles of PV deferral (keeps PE fed while ACT exps)
BF16 = mybir.dt.bfloat16
F32 = mybir.dt.float32
AF = mybir.ActivationFunctionType
ALU = mybir.AluOpType
NPBF16 = ml_dtypes.bfloat16
FP8 = mybir.dt.float8e4
NPFP8 = mybir.dt.np(FP8)
DR = mybir.MatmulPerfMode.DoubleRow
DTP = DT // 2        # 3 doubled d-tiles for the fp8 DoubleRow QK projection
WSCALE = 64.0        # fp8 weight prescale (keeps W_Q/W_K out of e4m3 subnormals)

# mechanism toggles
ACT_QEVAC = False    # Q^T evac on ACT via activation Identity+bias
PSUM_RECIP = False   # reciprocal_approx_fast reads denominators from PSUM
DVE_MASK = False     # triangular mask on DVE instead of gpsimd


def _build_nc():
    nc = bacc.Bacc(
        "TRN2", target_bir_lowering=False, debug=False, num_devices=N_CORES
    )
    xt_d = nc.dram_tensor("xt", [P, DT, S], BF16, kind="ExternalInput").ap()
    x8_d = nc.dram_tensor("x8", [P, DTP, 2, S], FP8, kind="ExternalInput").ap()
    wqk_d = nc.dram_tensor("wqk8", [P, NPAIR, 2, DTP, 2, P], FP8, kind="ExternalInput").ap()
    wv_d = nc.dram_tensor("wv", [P, DT, N * H], BF16, kind="ExternalInput").ap()
    wo_d = nc.dram_tensor("wo", [P, NPAIR, D], BF16, kind="ExternalInput").ap()
    bqk_d = nc.dram_tensor("bqk", [P, 2, NPAIR], F32, kind="ExternalInput").ap()
    bv_d = nc.dram_tensor("bv", [N * H], F32, kind="ExternalInput").ap()
    bo_d = nc.dram_tensor("bo", [D], F32, kind="ExternalInput").ap()
    out_d = nc.dram_tensor("out", [S, D], BF16, kind="ExternalOutput").ap()

    with tile.TileContext(nc) as tc, ExitStack() as ctx:
        _body(ctx, tc, xt_d, x8_d, wqk_d, wv_d, wo_d, bqk_d, bv_d, bo_d, out_d)
    nc.compile()
    return nc


def _body(ctx, tc, xt_d, x8_d, wqk_d, wv_d, wo_d, bqk_d, bv_d, bo_d, out_d):
    nc = tc.nc
    const = ctx.enter_context(tc.tile_pool(name="const", bufs=1))
    ppool = ctx.enter_context(tc.tile_pool(name="ppool", bufs=8))
    spool = ctx.enter_context(tc.tile_pool(name="spool", bufs=4))
    opool = ctx.enter_context(tc.tile_pool(name="opool", bufs=4))
    ps_mm = ctx.enter_context(tc.tile_pool(name="ps_mm", bufs=3, space="PSUM"))
    ps_pj = ps_mm
    ps_z = ctx.enter_context(tc.tile_pool(name="ps_z", bufs=2, space="PSUM"))

    # --- engine warmups ----------------------------------------------------
    # DVE pays ~11us on its first real op; ACT pays a ~2.7us exp-table load.
    # Absorb both at t=0, concurrent with the input DMAs.
    warm = const.tile([1, 8], F32, tag="warm")
    nc.vector.memset(warm[:], 1.0)
    warmp = ps_z.tile([1, 8], F32, tag="z", name="warmp")
    nc.vector.tensor_copy(warmp[:], warm[:])
    warmb = const.tile([1, 8], BF16, tag="warmb")
    nc.vector.tensor_copy(warmb[:], warmp[:])  # preload DVE psum-read CAST path
    nc.scalar.activation(warm[:], warm[:], AF.Exp, scale=1.0)

    # --- constants ---------------------------------------------------------
    # trimask[r, c] = 1 if r <= c else 0 (keep k <= q in [k, q] layout)
    trimask = const.tile([P, P], BF16, tag="trimask")
    make_upper_triangular(nc, trimask[:], val=1.0, diag=True)

    # --- input DMAs --------------------------------------------------------
    xt = const.tile([P, DT, S], BF16, tag="xt")
    x8 = const.tile([P, DTP, 2, S], FP8, tag="x8")
    wqk = const.tile([P, NPAIR, 2, DTP, 2, P], FP8, tag="wqk8")
    wv_sb = const.tile([P, DT, N * H], BF16, tag="wv")
    wo_sb = const.tile([P, NPAIR, D], BF16, tag="wo")
    bqk_sb = const.tile([P, 2, NPAIR], F32, tag="bqk")
    bv_rep = const.tile([P, N * H], F32, tag="bvrep")
    bo_rep = const.tile([P, D], F32, tag="borep")

    # gpsimd SW queue: biases first (tiny; bq/bk needed at qk0 evac), then wv
    nc.gpsimd.dma_start(bqk_sb[:], bqk_d)
    nc.gpsimd.dma_start(
        bv_rep[:], bv_d[None, :].to_broadcast((P, N * H))
    )
    for dt in range(DT):
        nc.gpsimd.dma_start(wv_sb[:, dt, :], wv_d[:, dt, :])
    nc.gpsimd.dma_start(bo_rep[:], bo_d[None, :].to_broadcast((P, D)))

    # scalar HW queue: pair-0 weights + last x8 chunk lead, then odd xt
    # chunks, then the remaining weights
    nc.scalar.dma_start(wqk[:, 0], wqk_d[:, 0])
    nc.scalar.dma_start(x8[:, 2], x8_d[:, 2])
    for dt in (1, 3, 5):
        nc.scalar.dma_start(xt[:, dt, :], xt_d[:, dt, :])
    for pr in range(1, NPAIR):
        nc.scalar.dma_start(wqk[:, pr], wqk_d[:, pr])
    nc.scalar.dma_start(wo_sb[:], wo_d)

    # sync HW queue: first x8 chunks, then even xt chunks (out tiles ride
    # this queue later)
    for dtp in (0, 1):
        nc.sync.dma_start(x8[:, dtp], x8_d[:, dtp])
    for dt in (0, 2, 4):
        nc.sync.dma_start(xt[:, dt, :], xt_d[:, dt, :])

    # --- persistent attention tiles ---------------------------------------
    qT = const.tile([P, NPAIR, S], BF16, tag="qT")
    kT = const.tile([P, NPAIR, S], BF16, tag="kT")
    zT = const.tile([P, NPAIR, S], BF16, tag="zT")
    # V_aug: the 64-wide ones block makes the PV matmul replicate the softmax
    # denominators into PSUM partitions 64:128 -- broadcast for free.  Only
    # the ones region needs the memset; v_proj overwrites the value region.
    v_aug = const.tile([P, KT, N, 2 * H], BF16, tag="vaug")
    nc.vector.memset(v_aug[:, :, :, H : 2 * H], 1.0)

    def qk_proj_steps(pr):
        # Q^T (bank 0) and K^T (bank 1) of one 2-bank psum tile, as a list of
        # single-matmul closures so the pair-(pr) projection can be drip-fed
        # into pair-(pr-1)'s attention stream as PE filler work.  Q evacuates
        # on ACT (activation+bias), K on DVE, splitting the evac load.
        steps = []
        for sb_i in range(SB):
            box = {}

            def mk(dtp, half, sb_i=sb_i, box=box):
                def go():
                    if "t" not in box:
                        box["t"] = ps_pj.tile(
                            [P, 2 * QB], F32, tag="mm", name=f"pqk_{pr}_{sb_i}"
                        )
                    pqk = box["t"]
                    nc.tensor.matmul(
                        pqk[:, half * QB : (half + 1) * QB],
                        lhsT=wqk[:, pr, half, dtp],
                        rhs=x8[:, dtp, :, bass.ts(sb_i, QB)],
                        start=(dtp == 0),
                        stop=(dtp == DTP - 1),
                        perf_mode=DR,
                    )
                    if half == 1 and dtp == DTP - 1:
                        if ACT_QEVAC:
                            nc.scalar.activation(
                                qT[:, pr, bass.ts(sb_i, QB)], pqk[:, 0:QB],
                                AF.Identity, bias=bqk_sb[:, 0, pr : pr + 1],
                                scale=1.0 / WSCALE,
                            )
                        else:
                            nc.vector.tensor_scalar(
                                qT[:, pr, bass.ts(sb_i, QB)], pqk[:, 0:QB],
                                1.0 / WSCALE, bqk_sb[:, 0, pr : pr + 1],
                                ALU.mult, ALU.add,
                            )
                        nc.vector.tensor_scalar(
                            kT[:, pr, bass.ts(sb_i, QB)], pqk[:, QB : 2 * QB],
                            1.0 / WSCALE, bqk_sb[:, 1, pr : pr + 1],
                            ALU.mult, ALU.add,
                        )

                return go

            for half in range(2):
                for dtp in range(DTP):
                    steps.append(mk(dtp, half))
        return steps

    def qk_proj_sb(pr, sb_i):
        for s in qk_proj_steps(pr)[sb_i * 2 * DTP : (sb_i + 1) * 2 * DTP]:
            s()

    # --- V projection: dt-major, two 4-s-tile phases ----------------------
    # 8 concurrent PSUM accumulation groups per phase (3 ps_mm tiles hosting
    # 2 384-wide groups each + 2 ps_z tiles hosting 1), so the first matmuls
    # need only wv[:, 0] off the wire instead of the whole weight.
    def v_proj_phase(stiles, cb=None):
        # 2 ps_mm + 2 ps_z tiles per phase: one ps_mm ring slot stays free
        # so `cb` can interleave pair-0 S^T tiles (their exps run on the
        # otherwise-idle ACT during the V projection)
        n_groups = 2 * len(stiles)
        n_mm = (n_groups - 2 + 1) // 2
        tiles = [
            ps_mm.tile([P, 2 * QB], F32, tag="mm", name=f"pv{stiles[0]}_{i}")
            for i in range(n_mm)
        ]
        ztiles = [
            ps_z.tile([P, QB], F32, tag="z", name=f"pvz{stiles[0]}_{i}")
            for i in range(2)
        ]
        regions = []
        for g in range(n_groups):
            st, blk = stiles[g // 2], g % 2
            if g < 2 * n_mm:
                reg = tiles[g // 2][:, (g % 2) * QB : (g % 2) * QB + EB]
            else:
                reg = ztiles[g - 2 * n_mm][:, :EB]
            regions.append((reg, st, blk))
        for dt in range(DT):
            for reg, st, blk in regions:
                nc.tensor.matmul(
                    reg,
                    lhsT=xt[:, dt, bass.ts(st, P)],
                    rhs=wv_sb[:, dt, bass.ts(blk, EB)],
                    start=(dt == 0),
                    stop=(dt == DT - 1),
                )
            if cb is not None and dt == 3:
                cb()
        for reg, st, blk in regions:
            nc.vector.tensor_tensor(
                v_aug[:, st, bass.ts(blk, 6), 0:H],
                reg.rearrange("p (n h) -> p n h", h=H),
                bv_rep[:, bass.ts(blk, EB)].rearrange("p (n h) -> p n h", h=H),
                ALU.add,
            )

    def o_proj_steps(qts, alt_pool=False):
        # out[q, e] = z^T.T @ W_O + b_O, as single-matmul closures
        steps = []
        for qt in qts:
            for eb in range(D // EB):
                box = {}

                def mk(pr, qt=qt, eb=eb, box=box):
                    def go():
                        if "t" not in box:
                            # tail-only: alternate into the attention z-pool
                            # (free after the last pair) for deeper rotation
                            if alt_pool and (2 * qt + eb) % 2:
                                box["t"] = ps_z.tile(
                                    [P, QB], F32, tag="z", name=f"po_{qt}_{eb}"
                                )
                            else:
                                box["t"] = ps_mm.tile(
                                    [P, 2 * QB], F32, tag="mm", name=f"po_{qt}_{eb}"
                                )
                        po = box["t"]
                        nc.tensor.matmul(
                            po[:, :EB],
                            lhsT=zT[:, pr, bass.ts(qt, P)],
                            rhs=wo_sb[:, pr, bass.ts(eb, EB)],
                            start=(pr == 0),
                            stop=(pr == NPAIR - 1),
                        )
                        if pr == NPAIR - 1:
                            ot = opool.tile([P, EB], BF16, tag="ot")
                            nc.vector.tensor_tensor(
                                ot[:], po[:, :EB], bo_rep[:, bass.ts(eb, EB)],
                                ALU.add,
                            )
                            oeng = (nc.sync, nc.scalar, nc.gpsimd)[(2 * qt + eb) % 3]
                            oeng.dma_start(
                                out_d[bass.ts(qt, P), bass.ts(eb, EB)], ot[:]
                            )

                    return go

                for pr in range(NPAIR):
                    steps.append(mk(pr))
        return steps

    o_first = o_proj_steps(range(4))  # q-tiles 0-3: fills attn(last, j=1)
    o_idx = [0]

    # --- attention block machinery ----------------------------------------

    class Block:
        def __init__(self, pr, j, final=False):
            self.pr, self.j = pr, j
            self.final = final
            self.n_kt = 4 * (j + 1)
            self.pz = None
            self.pts = {}
            self.normed = False

        def alloc_pz(self):
            if self.pz is None:
                self.pz = [
                    ps_z.tile([P, QB], F32, tag="z", name=f"z_{self.pr}_{self.j}_{h}")
                    for h in range(2)
                ]

        def emit_st(self, i):
            # S^T for both halves into one 2-bank tile; exp; mask
            pr, j = self.pr, self.j
            q_off = max(0, (i - 4 * j) * P)
            ps = ps_mm.tile([P, 2 * QB], F32, tag="mm", name=f"s_{pr}_{j}_{i}")
            for half in range(2):
                lo, hi = 64 * half, 64 * half + 64
                nc.tensor.matmul(
                    ps[:, half * QB + q_off : (half + 1) * QB],
                    lhsT=kT[lo:hi, pr, bass.ts(i, P)],
                    rhs=qT[lo:hi, pr, j * QB + q_off : (j + 1) * QB],
                    start=True,
                    stop=True,
                )
            pT = ppool.tile([P, 2, QB], BF16, tag="pT")
            ps3 = ps.rearrange("p (h q) -> p h q", h=2)
            nc.scalar.activation(
                pT[:, :, q_off:], ps3[:, :, q_off:], AF.Exp, scale=0.125
            )
            if i >= 4 * j:  # diagonal tile: triangular mask, both halves
                # final block: DVE (lower latency on the exposed tail chain)
                mask_eng = nc.vector if (DVE_MASK or self.final) else nc.gpsimd
                mask_eng.tensor_tensor(
                    pT[:, :, q_off : q_off + P],
                    pT[:, :, q_off : q_off + P],
                    trimask[:, None, :].to_broadcast((P, 2, P)),
                    ALU.mult,
                )
            self.pts[i] = pT

        def emit_pv(self, i):
            pr, j = self.pr, self.j
            self.alloc_pz()
            q_off = max(0, (i - 4 * j) * P)
            for half in range(2):
                n = 2 * pr + half
                nc.tensor.matmul(
                    self.pz[half][:, q_off:],
                    lhsT=v_aug[:, i, n, :],
                    rhs=self.pts[i][:, half, q_off:],
                    start=(i == 0),
                    stop=(i == self.n_kt - 1),
                )
            del self.pts[i]

        def emit_norm(self, merged=False):
            # normalize z and store z^T (PSUM rows 64:128 hold the
            # denominators replicated by the ones block).  merged=True packs
            # both halves into one reciprocal (5 DVE ops instead of 6) --
            # used for the final block, whose norm chain is fully exposed.
            pr, j = self.pr, self.j
            if merged:
                sm = spool.tile([P, QB], F32, tag="smm", name=f"smm_{pr}_{j}")
                for half in range(2):
                    nc.vector.tensor_copy(
                        sm[64 * half : 64 * half + 64, :],
                        self.pz[half][H : 2 * H, :],
                    )
                rc = spool.tile([P, QB], F32, tag="rcm", name=f"rcm_{pr}_{j}")
                nc.vector.reciprocal_approx_fast(rc[:], sm[:])
                for half in range(2):
                    lo, hi = 64 * half, 64 * half + 64
                    nc.vector.tensor_mul(
                        zT[lo:hi, pr, bass.ts(j, QB)],
                        self.pz[half][0:H, :], rc[lo:hi, :],
                    )
                self.normed = True
                return
            for half in range(2):
                lo, hi = 64 * half, 64 * half + 64
                if PSUM_RECIP:
                    den = self.pz[half][H : 2 * H, :]
                else:
                    sm = spool.tile([64, QB], F32, tag="sm")
                    nc.vector.tensor_copy(sm[:], self.pz[half][H : 2 * H, :])
                    den = sm[:]
                rc = spool.tile([64, QB], F32, tag="rc")
                nc.vector.reciprocal_approx_fast(rc[:], den)
                nc.vector.tensor_mul(
                    zT[lo:hi, pr, bass.ts(j, QB)], self.pz[half][0:H, :], rc[:]
                )
            self.normed = True

    # --- emission: qk0 | v phases (pair-0 S^T/exp interleaved) | pairs ----
    # Pair-0 j=0's S^T tiles ride the free ps_mm ring slot inside the V
    # projection: their exps run on the otherwise-idle ACT, so pair 0
    # opens with its P^T tiles already computed.
    qk_proj_sb(0, 0)
    qk_proj_sb(0, 1)
    b00 = Block(0, 0)
    st_cnt = [0]

    def _stcb():
        if st_cnt[0] < b00.n_kt:
            b00.emit_st(st_cnt[0])
            st_cnt[0] += 1

    v_proj_phase((0, 1, 2), _stcb)
    v_proj_phase((3, 4, 5), _stcb)
    v_proj_phase((6, 7), _stcb)
    _stcb()  # remaining pair-0 j=0 tile

    # v5 emission: pair-major, per-block loop, drip-fed filler
    for pr in range(NPAIR):
        last = pr + 1 >= NPAIR
        fill = qk_proj_steps(pr + 1) if not last else None
        fill_i = [0]

        def emit_fill(k=1):
            for _ in range(k):
                if fill is not None:
                    if fill_i[0] < len(fill):
                        fill[fill_i[0]]()
                        fill_i[0] += 1
                elif cur_j[0] == 1 and o_idx[0] < len(o_first):
                    o_first[o_idx[0]]()
                    o_idx[0] += 1

        cur_j = [0]
        for j in range(SB):
            cur_j[0] = j
            final = pr == NPAIR - 1 and j == SB - 1
            b = b00 if (pr == 0 and j == 0) else Block(pr, j, final=final)
            n_kt = b.n_kt
            # the final block runs a shorter PV lookahead so its drain ends
            # (and the fully-exposed last normalize starts) earlier
            la = 2 if final else LOOKAHEAD
            for i in range(n_kt):
                if not (b is b00 and i < st_cnt[0]):
                    b.emit_st(i)
                emit_fill(2 if i < 2 else 1)
                if i >= la:
                    b.emit_pv(i - la)
                    emit_fill()
            for i in range(max(0, n_kt - la), n_kt):
                b.emit_pv(i)
                if not final:
                    emit_fill()
            b.emit_norm(merged=final)

    # --- output projection leftovers -----------------------------------
    for s in o_first[o_idx[0] :]:
        s()
    for s in o_proj_steps(range(4, KT), alt_pool=True):
        s()


_CACHE = {}


def get_nc():
    if "nc" not in _CACHE:
        _CACHE["nc"] = _build_nc()
    return _CACHE["nc"]


def _prep_shared(W_Q, W_K, W_V, W_O, b_Q, b_K, b_V, b_O):
    W_Q = np.asarray(W_Q, np.float32)
    W_K = np.asarray(W_K, np.float32)
    W_V = np.asarray(W_V, np.float32)
    W_O = np.asarray(W_O, np.float32)
    # wqk8 [dp, pr, half, dtp, j, a*64+h], prescaled by WSCALE
    def qk_img(W):
        return (W * WSCALE).reshape(NPAIR, 2, DTP, 2, P, H).transpose(4, 0, 2, 3, 1, 5)
    wqk = np.stack([qk_img(W_Q), qk_img(W_K)], axis=2).reshape(
        P, NPAIR, 2, DTP, 2, P
    ).astype(NPFP8)
    wv = np.ascontiguousarray(
        W_V.reshape(N, DT, P, H).transpose(2, 1, 0, 3).reshape(P, DT, N * H)
    ).astype(NPBF16)
    wo = np.ascontiguousarray(
        W_O.reshape(NPAIR, 2, H, D).transpose(1, 2, 0, 3).reshape(P, NPAIR, D)
    ).astype(NPBF16)
    def b_img(b):
        return np.asarray(b, np.float32).reshape(NPAIR, 2, H).transpose(1, 2, 0).reshape(P, NPAIR)
    bqk = np.ascontiguousarray(
        np.stack([b_img(b_Q), b_img(b_K)], axis=1)
    ).astype(np.float32)
    return {
        "wqk8": np.ascontiguousarray(wqk),
        "wv": wv,
        "wo": wo,
        "bqk": bqk,
        "bv": np.ascontiguousarray(np.asarray(b_V, np.float32).reshape(N * H)),
        "bo": np.ascontiguousarray(np.asarray(b_O, np.float32)),
    }


def _prep_x(xb):
    # xt: [1024, 768] f32 -> [128, 6, 1024] bf16 (partition = d%128)
    # x8: same data as fp8, d-tiles pair-interleaved for DoubleRow
    xT = xb.T.reshape(DT, P, S)
    xt = np.ascontiguousarray(xT.transpose(1, 0, 2)).astype(NPBF16)
    x8 = np.ascontiguousarray(
        xT.reshape(DTP, 2, P, S).transpose(2, 0, 1, 3)
    ).astype(NPFP8)
    return {"xt": xt, "x8": x8}


def kernel(normalized_resid_pre, W_Q, W_K, W_V, W_O, b_Q, b_K, b_V, b_O, **kw):
    x = np.asarray(normalized_resid_pre, dtype=np.float32)
    shared = _prep_shared(W_Q, W_K, W_V, W_O, b_Q, b_K, b_V, b_O)
    in_maps = [dict(shared, **_prep_x(x[b])) for b in range(B)]
    nc = get_nc()
    res = run_bass_kernel_spmd(nc, in_maps, core_ids=list(range(N_CORES)))
    return np.stack(
        [np.asarray(res.results[b]["out"], np.float32) for b in range(B)], axis=0
    )
